# revision 5
# baseline (speedup 1.0000x reference)
"""DISCO S2 conv (DiscreteContinuousConvS2) Trainium2 Bass kernel.

Algorithm (validated vs reference in float64):
  The sparse psi tensor applied with 360 longitude shifts is a circular
  correlation along longitude.  psi is exactly even in longitude offset, so
  its longitude-DFT is purely real.  Pipeline per core:
    1. einsum over C_in fused with layout transpose:  xwT[po, m] = x[:,la,po].T @ w2
    2. forward rDFT over longitude as a matmul with a precomputed [360,362]
       cos/-sin matrix (stacked re/im), contracting po on the partition dim
    3. per-(k,dla) diagonal spectral multiply-accumulate on the Vector engine
       (14 nonzero (k,dla) pairs, P-hat broadcast over output channels)
    4. inverse rDFT as a matmul with a precomputed [362,360] matrix
  Sharding: 8 cores = (batch b in 0..3) x (C_out half), fully data-parallel,
  no collectives.  Latitude processed in three ho-bands with +-4 la halo.
"""
import sys
import numpy as np

for _p in ("/opt/trn_rl_repo",):
    if _p not in sys.path:
        sys.path.insert(0, _p)

NLAT, NLON, NF, FDIM = 181, 360, 181, 362
K, B, CIN, COUT, OH = 2, 4, 96, 96, 48
THIRDS = [(0, 61), (61, 121), (121, 181)]
FS = [(0, 128), (128, 256), (256, 362)]
PS = [(0, 128), (128, 256), (256, 360)]
NZ = [(1, 0), (0, -2), (0, -1), (0, 0), (0, 1), (0, 2),
      (1, -4), (1, -3), (1, -2), (1, -1), (1, 1), (1, 2), (1, 3), (1, 4)]
NPAIR = len(NZ)

_CACHE = {}


def _host_prep(weight, psi_vals, k_idx, ho_idx, lat_in, lon_in):
    dla_all = lat_in.astype(np.int64) - ho_idx.astype(np.int64)
    P = np.zeros((K, 9, NLAT, NLON), dtype=np.float64)
    np.add.at(P, (k_idx, dla_all + 4, ho_idx, lon_in), psi_vals.astype(np.float64))
    f = np.arange(NF)
    ang = 2 * np.pi * np.outer(np.arange(NLON), f) / NLON          # [360,181]
    dfwd = np.concatenate([np.cos(ang), -np.sin(ang)], axis=1).astype(np.float32)
    cf = np.full(NF, 2.0 / NLON)
    cf[0] = 1.0 / NLON
    cf[NF - 1] = 1.0 / NLON
    dinv = np.concatenate([cf[:, None] * np.cos(ang.T),
                           -cf[:, None] * np.sin(ang.T)], axis=0)
    dinv[NF, :] = 0.0
    dinv[2 * NF - 1, :] = 0.0
    dinv = np.ascontiguousarray(dinv.astype(np.float32))           # [362,360]
    phat_all = P @ np.cos(ang)                                     # [K,9,NLAT,181]
    phat = np.zeros((NPAIR, FDIM, NLAT), dtype=np.float32)
    for ip, (k, dla) in enumerate(NZ):
        pT = phat_all[k, dla + 4].T.astype(np.float32)             # [181f,181ho]
        phat[ip, :NF] = pT
        phat[ip, NF:] = pT
    return np.ascontiguousarray(dfwd), dinv, phat


def _build_nc():
    import concourse.bass as bass
    import concourse.bacc as bacc
    import concourse.tile as tile
    from concourse import mybir

    f32 = mybir.dt.float32
    f32r = mybir.dt.float32r

    nc = bacc.Bacc("TRN2", target_bir_lowering=False, debug=False)

    x_in = nc.dram_tensor("x_in", [CIN, NLAT, NLON], f32, kind="ExternalInput").ap()
    w2_in = nc.dram_tensor("w2_in", [CIN, OH * K], f32, kind="ExternalInput").ap()
    dfwd_in = nc.dram_tensor("dfwd_in", [NLON, FDIM], f32, kind="ExternalInput").ap()
    dinv_in = nc.dram_tensor("dinv_in", [FDIM, NLON], f32, kind="ExternalInput").ap()
    phat_in = nc.dram_tensor("phat_in", [NPAIR, FDIM, NLAT], f32, kind="ExternalInput").ap()
    out_d = nc.dram_tensor("out", [OH * NLAT, NLON], f32, kind="ExternalOutput").ap()

    from contextlib import ExitStack
    with tile.TileContext(nc) as tc, ExitStack() as es:
        consts = es.enter_context(tc.tile_pool(name="consts", bufs=1))
        phat_pool = es.enter_context(tc.tile_pool(name="phat", bufs=2))
        xh_pool = es.enter_context(tc.tile_pool(name="xh", bufs=1))
        yh_pool = es.enter_context(tc.tile_pool(name="yh", bufs=1))
        small = es.enter_context(tc.tile_pool(name="small", bufs=3))
        tmp_pool = es.enter_context(tc.tile_pool(name="tmp", bufs=3))
        ps_a = es.enter_context(tc.tile_pool(name="ps_a", bufs=2, space=bass.MemorySpace.PSUM))
        ps_b = es.enter_context(tc.tile_pool(name="ps_b", bufs=2, space=bass.MemorySpace.PSUM))
        ps_e = es.enter_context(tc.tile_pool(name="ps_e", bufs=2, space=bass.MemorySpace.PSUM))

        w2_sb = consts.tile([CIN, OH * K], f32r)
        nc.sync.dma_start(out=w2_sb[:, :], in_=w2_in[:, :].bitcast(f32r))
        dfwd_sb = consts.tile([128, 3, FDIM], f32r)
        for j, (p0, p1) in enumerate(PS):
            nc.sync.dma_start(out=dfwd_sb[:p1 - p0, j, :], in_=dfwd_in[p0:p1, :].bitcast(f32r))
        dinv_sb = consts.tile([128, 3, NLON], f32r)
        for t, (f0, f1) in enumerate(FS):
            nc.sync.dma_start(out=dinv_sb[:f1 - f0, t, :], in_=dinv_in[f0:f1, :].bitcast(f32r))

        for (ho_lo, ho_hi) in THIRDS:
            la_lo = max(0, ho_lo - 4)
            la_hi = min(NLAT, ho_hi + 4)
            nla_w = la_hi - la_lo
            how = ho_hi - ho_lo

            phat_sb = phat_pool.tile([128, 3, NPAIR, how], f32, tag="phat")
            for t, (f0, f1) in enumerate(FS):
                for ip in range(NPAIR):
                    nc.sync.dma_start(
                        out=phat_sb[:f1 - f0, t, ip, :],
                        in_=phat_in[ip, f0:f1, ho_lo:ho_hi])

            xh = xh_pool.tile([128, 3, OH, K, nla_w], f32, tag="xh")

            # ---- stages A+B: einsum-T then forward DFT ----
            for g in range(0, nla_w, 4):
                nla = min(4, nla_w - g)
                la0 = la_lo + g
                x_t = small.tile([CIN, 4, NLON], f32r, tag="x_t")
                nc.sync.dma_start(out=x_t[:, :nla, :], in_=x_in[:, la0:la0 + nla, :].bitcast(f32r))
                xwT = small.tile([128, 3, 4, OH * K], f32r, tag="xwT")
                for il in range(nla):
                    for j, (p0, p1) in enumerate(PS):
                        pc = p1 - p0
                        ps_t = ps_a.tile([128, OH * K], f32, tag="ps_a")
                        nc.tensor.matmul(
                            ps_t[:pc, :],
                            x_t[:, il, p0:p1],
                            w2_sb[:, :],
                            start=True, stop=True)
                        nc.any.tensor_copy(xwT[:pc, j, il, :], ps_t[:pc, :])
                for t, (f0, f1) in enumerate(FS):
                    fsz = f1 - f0
                    ps_f = ps_b.tile([128, 4, OH, K], f32, tag="ps_b")
                    for j, (p0, p1) in enumerate(PS):
                        pc = p1 - p0
                        nc.tensor.matmul(
                            ps_f[:fsz, :nla, :, :],
                            dfwd_sb[:pc, j, f0:f1],
                            xwT[:pc, j, :nla, :],
                            start=(j == 0), stop=(j == 2))
                    nc.any.tensor_copy(
                        xh[:fsz, t, :, :, g:g + nla],
                        ps_f[:fsz, :nla, :, :].transpose([0, 2, 3, 1]))

            # ---- stage D: spectral multiply-accumulate ----
            yh = yh_pool.tile([128, 3, OH, how], f32r, tag="yh")
            for ip, (k, dla) in enumerate(NZ):
                ho0 = max(ho_lo, -dla)
                ho1 = min(ho_hi, NLAT - dla)
                w = ho1 - ho0
                if w <= 0:
                    continue
                a = ho0 + dla - la_lo
                hl = ho0 - ho_lo
                for t, (f0, f1) in enumerate(FS):
                    fsz = f1 - f0
                    for oc in range(2):
                        o0 = oc * 24
                        xs = xh[:fsz, t, o0:o0 + 24, k, a:a + w]
                        pb = phat_sb[:fsz, t, ip, hl:hl + w]
                        pbc = bass.AP(
                            tensor=pb.tensor, offset=pb.offset,
                            ap=[list(pb.ap[0]), [0, 24], list(pb.ap[1])])
                        if ip == 0:
                            nc.vector.tensor_mul(
                                yh[:fsz, t, o0:o0 + 24, hl:hl + w], xs, pbc)
                        else:
                            tm = tmp_pool.tile([128, 24, how], f32, tag="tmp")
                            nc.vector.tensor_mul(tm[:fsz, :, :w], xs, pbc)
                            nc.vector.tensor_add(
                                yh[:fsz, t, o0:o0 + 24, hl:hl + w],
                                yh[:fsz, t, o0:o0 + 24, hl:hl + w],
                                tm[:fsz, :, :w])

            # ---- stage E: inverse DFT + store ----
            for o in range(OH):
                ps_o = ps_e.tile([64, NLON], f32, tag="ps_e")
                for t, (f0, f1) in enumerate(FS):
                    fsz = f1 - f0
                    nc.tensor.matmul(
                        ps_o[:how, :],
                        yh[:fsz, t, o, :],
                        dinv_sb[:fsz, t, :],
                        start=(t == 0), stop=(t == 2))
                o_sb = small.tile([64, NLON], f32, tag="o_sb")
                nc.any.tensor_copy(o_sb[:how, :], ps_o[:how, :])
                r0 = o * NLAT + ho_lo
                nc.sync.dma_start(out=out_d[r0:r0 + how, :], in_=o_sb[:how, :])

    nc.compile()
    return nc


def kernel(x, weight, bias, psi_vals, k_idx, ho_idx, lat_in_idx, lon_in_idx):
    from concourse.bass_utils import run_bass_kernel_spmd

    x = np.ascontiguousarray(np.asarray(x, dtype=np.float32))
    weight = np.asarray(weight, dtype=np.float32)
    bias = np.asarray(bias, dtype=np.float32)
    dfwd, dinv, phat = _host_prep(
        weight, np.asarray(psi_vals), np.asarray(k_idx), np.asarray(ho_idx),
        np.asarray(lat_in_idx), np.asarray(lon_in_idx))

    if "nc" not in _CACHE:
        _CACHE["nc"] = _build_nc()
    nc = _CACHE["nc"]

    in_maps = []
    for s in range(8):
        b, ohf = s // 2, s % 2
        o_sl = slice(OH * ohf, OH * ohf + OH)
        w2 = np.ascontiguousarray(
            weight[o_sl].transpose(1, 0, 2).reshape(CIN, OH * K))
        in_maps.append({
            "x_in": x[b],
            "w2_in": w2,
            "dfwd_in": dfwd,
            "dinv_in": dinv,
            "phat_in": phat,
        })

    import os
    trace = bool(int(os.environ.get("KERNEL_TRACE", "0")))
    res = run_bass_kernel_spmd(nc, in_maps, core_ids=list(range(8)), trace=trace)
    _CACHE["last_res"] = res
    out = np.empty((B, COUT, NLAT, NLON), dtype=np.float32)
    for s in range(8):
        b, ohf = s // 2, s % 2
        out[b, OH * ohf:OH * ohf + OH] = res.results[s]["out"].reshape(OH, NLAT, NLON)
    if np.any(bias):
        out += bias[None, :, None, None]
    return out


# revision 7
# speedup vs baseline: 679.0949x; 679.0949x over previous
"""DISCO S2 conv (DiscreteContinuousConvS2) Trainium2 Bass kernel.

Algorithm (validated vs reference in float64):
  The sparse psi tensor applied with 360 longitude shifts is a circular
  correlation along longitude.  psi is exactly even in longitude offset, so
  its longitude-DFT is purely real.  Pipeline per core:
    1. einsum over C_in fused with layout transpose:  xwT[po, m] = x[:,la,po].T @ w2
    2. forward rDFT over longitude as a matmul with a precomputed [360,362]
       cos/-sin matrix (stacked re/im), contracting po on the partition dim
    3. per-(k,dla) diagonal spectral multiply-accumulate on the Vector engine
       (14 nonzero (k,dla) pairs, P-hat broadcast over output channels)
    4. inverse rDFT as a matmul with a precomputed [362,360] matrix
  Sharding: 8 cores = (batch b in 0..3) x (C_out half), fully data-parallel,
  no collectives.  Latitude processed in three ho-bands with +-4 la halo.
"""
import sys
import numpy as np

for _p in ("/opt/trn_rl_repo",):
    if _p not in sys.path:
        sys.path.insert(0, _p)

NLAT, NLON, NF, FDIM = 181, 360, 181, 362
K, B, CIN, COUT, OH = 2, 4, 96, 96, 48
THIRDS = [(0, 61), (61, 121), (121, 181)]
FS = [(0, 128), (128, 256), (256, 362)]
PS = [(0, 128), (128, 256), (256, 360)]
NZ = [(1, 0), (0, -2), (0, -1), (0, 0), (0, 1), (0, 2),
      (1, -4), (1, -3), (1, -2), (1, -1), (1, 1), (1, 2), (1, 3), (1, 4)]
NPAIR = len(NZ)

_CACHE = {}


def _host_prep(weight, psi_vals, k_idx, ho_idx, lat_in, lon_in):
    dla_all = lat_in.astype(np.int64) - ho_idx.astype(np.int64)
    P = np.zeros((K, 9, NLAT, NLON), dtype=np.float64)
    np.add.at(P, (k_idx, dla_all + 4, ho_idx, lon_in), psi_vals.astype(np.float64))
    f = np.arange(NF)
    ang = 2 * np.pi * np.outer(np.arange(NLON), f) / NLON          # [360,181]
    dfwd = np.concatenate([np.cos(ang), -np.sin(ang)], axis=1).astype(np.float32)
    cf = np.full(NF, 2.0 / NLON)
    cf[0] = 1.0 / NLON
    cf[NF - 1] = 1.0 / NLON
    dinv = np.concatenate([cf[:, None] * np.cos(ang.T),
                           -cf[:, None] * np.sin(ang.T)], axis=0)
    dinv[NF, :] = 0.0
    dinv[2 * NF - 1, :] = 0.0
    dinv = np.ascontiguousarray(dinv.astype(np.float32))           # [362,360]
    phat_all = P @ np.cos(ang)                                     # [K,9,NLAT,181]
    phat = np.zeros((NPAIR, FDIM, NLAT), dtype=np.float32)
    for ip, (k, dla) in enumerate(NZ):
        pT = phat_all[k, dla + 4].T.astype(np.float32)             # [181f,181ho]
        phat[ip, :NF] = pT
        phat[ip, NF:] = pT
    return np.ascontiguousarray(dfwd), dinv, phat


def _build_nc():
    import concourse.bass as bass
    import concourse.bacc as bacc
    import concourse.tile as tile
    from concourse import mybir

    f32 = mybir.dt.float32
    f32r = mybir.dt.float32r

    nc = bacc.Bacc("TRN2", target_bir_lowering=False, debug=False)

    x_in = nc.dram_tensor("x_in", [CIN, NLAT, NLON], f32, kind="ExternalInput").ap()
    w2_in = nc.dram_tensor("w2_in", [CIN, OH * K], f32, kind="ExternalInput").ap()
    dfwd_in = nc.dram_tensor("dfwd_in", [NLON, FDIM], f32, kind="ExternalInput").ap()
    dinv_in = nc.dram_tensor("dinv_in", [FDIM, NLON], f32, kind="ExternalInput").ap()
    phat_in = nc.dram_tensor("phat_in", [NPAIR, FDIM, NLAT], f32, kind="ExternalInput").ap()
    out_d = nc.dram_tensor("out", [OH * NLAT, NLON], f32, kind="ExternalOutput").ap()

    from contextlib import ExitStack
    with tile.TileContext(nc) as tc, ExitStack() as es:
        consts = es.enter_context(tc.tile_pool(name="consts", bufs=1))
        phat_pool = es.enter_context(tc.tile_pool(name="phat", bufs=2))
        xh_pool = es.enter_context(tc.tile_pool(name="xh", bufs=1))
        yh_pool = es.enter_context(tc.tile_pool(name="yh", bufs=1))
        small = es.enter_context(tc.tile_pool(name="small", bufs=3))
        tmp_pool = es.enter_context(tc.tile_pool(name="tmp", bufs=3))
        ps_a = es.enter_context(tc.tile_pool(name="ps_a", bufs=2, space=bass.MemorySpace.PSUM))
        ps_b = es.enter_context(tc.tile_pool(name="ps_b", bufs=2, space=bass.MemorySpace.PSUM))
        ps_e = es.enter_context(tc.tile_pool(name="ps_e", bufs=2, space=bass.MemorySpace.PSUM))

        w2_sb = consts.tile([CIN, OH * K], f32r)
        nc.sync.dma_start(out=w2_sb[:, :], in_=w2_in[:, :].bitcast(f32r))
        dfwd_sb = consts.tile([128, 3, FDIM], f32r)
        for j, (p0, p1) in enumerate(PS):
            nc.sync.dma_start(out=dfwd_sb[:p1 - p0, j, :], in_=dfwd_in[p0:p1, :].bitcast(f32r))
        dinv_sb = consts.tile([128, 3, NLON], f32r)
        for t, (f0, f1) in enumerate(FS):
            nc.sync.dma_start(out=dinv_sb[:f1 - f0, t, :], in_=dinv_in[f0:f1, :].bitcast(f32r))

        for (ho_lo, ho_hi) in THIRDS:
            la_lo = max(0, ho_lo - 4)
            la_hi = min(NLAT, ho_hi + 4)
            nla_w = la_hi - la_lo
            how = ho_hi - ho_lo

            phat_sb = phat_pool.tile([128, 3, NPAIR, how], f32, tag="phat")
            for t, (f0, f1) in enumerate(FS):
                for ip in range(NPAIR):
                    nc.sync.dma_start(
                        out=phat_sb[:f1 - f0, t, ip, :],
                        in_=phat_in[ip, f0:f1, ho_lo:ho_hi])

            xh = xh_pool.tile([128, 3, OH, K, nla_w], f32, tag="xh")

            # ---- stages A+B: einsum-T then forward DFT ----
            for g in range(0, nla_w, 4):
                nla = min(4, nla_w - g)
                la0 = la_lo + g
                x_t = small.tile([CIN, 4, NLON], f32r, tag="x_t")
                nc.sync.dma_start(out=x_t[:, :nla, :], in_=x_in[:, la0:la0 + nla, :].bitcast(f32r))
                xwT = small.tile([128, 3, 4, OH * K], f32r, tag="xwT")
                for il in range(nla):
                    for j, (p0, p1) in enumerate(PS):
                        pc = p1 - p0
                        ps_t = ps_a.tile([128, OH * K], f32, tag="ps_a")
                        nc.tensor.matmul(
                            ps_t[:pc, :],
                            x_t[:, il, p0:p1],
                            w2_sb[:, :],
                            start=True, stop=True)
                        nc.any.tensor_copy(xwT[:pc, j, il, :], ps_t[:pc, :])
                for t, (f0, f1) in enumerate(FS):
                    fsz = f1 - f0
                    ps_f = ps_b.tile([128, 4, OH, K], f32, tag="ps_b")
                    for j, (p0, p1) in enumerate(PS):
                        pc = p1 - p0
                        nc.tensor.matmul(
                            ps_f[:fsz, :nla, :, :],
                            dfwd_sb[:pc, j, f0:f1],
                            xwT[:pc, j, :nla, :],
                            start=(j == 0), stop=(j == 2))
                    nc.any.tensor_copy(
                        xh[:fsz, t, :, :, g:g + nla],
                        ps_f[:fsz, :nla, :, :].transpose([0, 2, 3, 1]))

            # ---- stage D: spectral multiply-accumulate ----
            yh = yh_pool.tile([128, 3, OH, how], f32r, tag="yh")
            for ip, (k, dla) in enumerate(NZ):
                ho0 = max(ho_lo, -dla)
                ho1 = min(ho_hi, NLAT - dla)
                w = ho1 - ho0
                if w <= 0:
                    continue
                a = ho0 + dla - la_lo
                hl = ho0 - ho_lo
                for t, (f0, f1) in enumerate(FS):
                    fsz = f1 - f0
                    for oc in range(2):
                        o0 = oc * 24
                        xs = xh[:fsz, t, o0:o0 + 24, k, a:a + w]
                        pb = phat_sb[:fsz, t, ip, hl:hl + w]
                        pbc = bass.AP(
                            tensor=pb.tensor, offset=pb.offset,
                            ap=[list(pb.ap[0]), [0, 24], list(pb.ap[1])])
                        if ip == 0:
                            nc.vector.tensor_mul(
                                yh[:fsz, t, o0:o0 + 24, hl:hl + w], xs, pbc)
                        else:
                            tm = tmp_pool.tile([128, 24, how], f32, tag="tmp")
                            nc.vector.tensor_mul(tm[:fsz, :, :w], xs, pbc)
                            nc.vector.tensor_add(
                                yh[:fsz, t, o0:o0 + 24, hl:hl + w],
                                yh[:fsz, t, o0:o0 + 24, hl:hl + w],
                                tm[:fsz, :, :w])

            # ---- stage E: inverse DFT + store ----
            for o in range(OH):
                ps_o = ps_e.tile([64, NLON], f32, tag="ps_e")
                for t, (f0, f1) in enumerate(FS):
                    fsz = f1 - f0
                    nc.tensor.matmul(
                        ps_o[:how, :],
                        yh[:fsz, t, o, :],
                        dinv_sb[:fsz, t, :],
                        start=(t == 0), stop=(t == 2))
                o_sb = small.tile([64, NLON], f32, tag="o_sb")
                nc.any.tensor_copy(o_sb[:how, :], ps_o[:how, :])
                r0 = o * NLAT + ho_lo
                nc.sync.dma_start(out=out_d[r0:r0 + how, :], in_=o_sb[:how, :])

    nc.compile()
    return nc


def _get_runner(n_cores=8):
    """Build (once) a jitted shard_map runner for the compiled Bass module.

    Mirrors concourse.bass2jax.run_bass_via_pjrt but caches the jitted
    callable so repeated kernel() calls skip retracing, and allocates the
    donated output buffers on-device instead of shipping host zeros.
    """
    if "runner" in _CACHE:
        return _CACHE["runner"]
    import jax
    import jax.numpy as jnp
    from jax.sharding import Mesh, PartitionSpec, NamedSharding
    from jax.experimental.shard_map import shard_map
    from concourse import bass2jax, mybir

    if "nc" not in _CACHE:
        _CACHE["nc"] = _build_nc()
    nc = _CACHE["nc"]
    bass2jax.install_neuronx_cc_hook()

    partition_name = (nc.partition_id_tensor.name
                      if nc.partition_id_tensor else None)
    in_names, out_names, out_avals = [], [], []
    for alloc in nc.m.functions[0].allocations:
        if not isinstance(alloc, mybir.MemoryLocationSet):
            continue
        name = alloc.memorylocations[0].name
        if alloc.kind == "ExternalInput":
            if name != partition_name:
                in_names.append(name)
        elif alloc.kind == "ExternalOutput":
            out_names.append(name)
            out_avals.append(jax.core.ShapedArray(
                tuple(alloc.tensor_shape), mybir.dt.np(alloc.dtype)))
    n_params = len(in_names)
    n_outs = len(out_avals)
    all_names = in_names + out_names
    if partition_name is not None:
        all_names = all_names + [partition_name]

    def _body(*args):
        operands = list(args)
        if partition_name is not None:
            operands.append(bass2jax.partition_id_tensor())
        outs = bass2jax._bass_exec_p.bind(
            *operands,
            out_avals=tuple(out_avals),
            in_names=tuple(all_names),
            out_names=tuple(out_names),
            lowering_input_output_aliases=(),
            sim_require_finite=True,
            sim_require_nnan=True,
            nc=nc,
        )
        return tuple(outs)

    devices = jax.devices()[:n_cores]
    mesh = Mesh(np.asarray(devices), ("core",))
    spec = PartitionSpec("core")
    sharding = NamedSharding(mesh, spec)
    donate = tuple(range(n_params, n_params + n_outs))
    sharded = jax.jit(
        shard_map(_body, mesh=mesh, in_specs=(spec,) * (n_params + n_outs),
                  out_specs=(spec,) * n_outs, check_rep=False),
        donate_argnums=donate, keep_unused=True)
    zero_shapes = [(n_cores * a.shape[0], *a.shape[1:]) for a in out_avals]
    zero_dtypes = [a.dtype for a in out_avals]
    make_zeros = jax.jit(
        lambda: tuple(jnp.zeros(s, d) for s, d in zip(zero_shapes, zero_dtypes)),
        out_shardings=(sharding,) * n_outs)
    runner = {
        "sharded": sharded, "make_zeros": make_zeros, "sharding": sharding,
        "in_names": in_names, "out_names": out_names, "out_avals": out_avals,
        "n_cores": n_cores,
    }
    _CACHE["runner"] = runner
    return runner


def _device_inputs(x, weight, psi_arrays):
    """Concatenated-global per-parameter arrays, device_put with sharding."""
    import jax
    dfwd, dinv, phat = _host_prep(weight, *psi_arrays)
    per_core = {"x_in": [], "w2_in": [], "dfwd_in": [], "dinv_in": [], "phat_in": []}
    for s in range(8):
        b, ohf = s // 2, s % 2
        o_sl = slice(OH * ohf, OH * ohf + OH)
        w2 = np.ascontiguousarray(
            weight[o_sl].transpose(1, 0, 2).reshape(CIN, OH * K))
        per_core["x_in"].append(x[b])
        per_core["w2_in"].append(w2)
        per_core["dfwd_in"].append(dfwd)
        per_core["dinv_in"].append(dinv)
        per_core["phat_in"].append(phat)
    runner = _get_runner()
    concat = {k: np.concatenate(v, axis=0) for k, v in per_core.items()}
    return [jax.device_put(concat[name], runner["sharding"])
            for name in runner["in_names"]]


def _run_device(dev_in):
    runner = _get_runner()
    zeros = runner["make_zeros"]()
    return runner["sharded"](*dev_in, *zeros)


def kernel(x, weight, bias, psi_vals, k_idx, ho_idx, lat_in_idx, lon_in_idx):
    x = np.ascontiguousarray(np.asarray(x, dtype=np.float32))
    weight = np.asarray(weight, dtype=np.float32)
    bias = np.asarray(bias, dtype=np.float32)
    psi_arrays = (np.asarray(psi_vals), np.asarray(k_idx), np.asarray(ho_idx),
                  np.asarray(lat_in_idx), np.asarray(lon_in_idx))

    dev_in = _device_inputs(x, weight, psi_arrays)
    out_arrs = _run_device(dev_in)
    runner = _get_runner()
    a0 = runner["out_avals"][0]
    res0 = np.asarray(out_arrs[0]).reshape(8, *a0.shape)

    out = np.empty((B, COUT, NLAT, NLON), dtype=np.float32)
    for s in range(8):
        b, ohf = s // 2, s % 2
        out[b, OH * ohf:OH * ohf + OH] = res0[s].reshape(OH, NLAT, NLON)
    if np.any(bias):
        out += bias[None, :, None, None]
    return out
